# revision 24
# baseline (speedup 1.0000x reference)
"""Trainium2 Bass kernel for nn_DifferentiableHistogram.

reference:
    px      = x.transpose(0,2,3,1).reshape(B, N, 3)           # B=4, N=65536
    dist_sq = ||px - c||^2  for K=512 bin centers             # (B,N,K)
    w       = exp(-dist_sq / (2*0.02^2))                      # = exp(-1250*d^2)
    w       = w / (sum_k w + 1e-8)
    hist    = sum_n w;  hist = hist / (sum_k hist + 1e-8)     # (B,K)

Sharding: 8 cores = 4 batches x 2 image-row halves. Each core computes a
partial unnormalized histogram over its 32768 pixels (per-pixel softmax
weights are independent across pixels); host sums the two halves per batch
and applies the final K-normalization.

Two device programs:
  * separable — when bin_centers is a 8x8x8 meshgrid (the reference's
    setup_inputs), exp factorizes per axis: w = wx_i * wy_j * wz_l. Only
    24 exps/pixel instead of 512, and the histogram becomes a 3-way
    contraction done on the tensor engine.
  * dense — any bin_centers: dist^2 via matmul (contract over the
    4-vector [x,y,z,1]), exp+row-sum fused on the scalar engine
    (accum_out), per-pixel 1/(S+1e-8) on vector, histogram accumulation
    as a second matmul contracting over pixels.
"""

import math

import numpy as np

import concourse.bass as bass
import concourse.tile as tile
from concourse import bacc, mybir
from concourse.bass_utils import run_bass_kernel_spmd

F32 = mybir.dt.float32
BF16 = mybir.dt.bfloat16
AF = mybir.ActivationFunctionType
ALU = mybir.AluOpType

B, C, H, W = 4, 3, 256, 256
BINS = 8
K = BINS**3
SIGMA = 0.02
LAM = 1.0 / (2.0 * SIGMA**2)  # 1250.0
N_CORES = 8
HH = H // 2          # rows per core: 128
NPIX = HH * W        # pixels per core: 32768
NT = W               # pixel tiles per core (one image column each): 256


def _make_nc():
    return bacc.Bacc("TRN2", target_bir_lowering=False, debug=False)


# --------------------------------------------------------------------------
# separable program: bin_centers = meshgrid(ax, ay, az), K = 8*8*8
# --------------------------------------------------------------------------
def _build_separable(ax, ay, az):
    nc = _make_nc()
    img = nc.dram_tensor("img", [C, HH, W], F32, kind="ExternalInput")
    cbias = nc.dram_tensor("cbias", [1, 4 * BINS], F32, kind="ExternalInput")
    hist_out = nc.dram_tensor("hist", [HH, BINS * BINS], F32,
                              kind="ExternalOutput")

    NCHB = 8                   # chunks over image columns
    FCHB = W // NCHB
    NPACK = 4                  # column-packed concurrent matmuls
    s = math.sqrt(LAM)
    axes = [ax, ay, az]

    with tile.TileContext(nc) as tc:
        with (
            tc.tile_pool(name="const", bufs=1) as const,
            tc.tile_pool(name="work", bufs=1) as work,
            tc.tile_pool(name="psum", bufs=1, space="PSUM") as psum,
        ):
            # bias columns first ([c*8+i] = -s*a_i for the Square path),
            # then a tiny Square to pull the ACT table load forward.
            cb = const.tile([HH, 4 * BINS], F32, tag="cb")
            cb_src = bass.AP(
                tensor=cbias.ap().tensor, offset=0,
                ap=[[0, HH], [1, 4 * BINS]],
            )
            nc.sync.dma_start(cb[:], cb_src)
            warm = work.tile([HH, 1], F32, tag="warm")
            nc.scalar.activation(warm[:], cb[:, 0:1], AF.Square)
            hs = work.tile([HH, BINS * BINS], F32, tag="hs")
            nc.vector.memset(hs[:], 0.0)

            # channels in image layout (rows x cols), split across queues
            X = [None] * C
            for c, eng in ((1, nc.sync), (0, nc.gpsimd), (2, nc.sync)):
                xc = const.tile([HH, W], F32, tag=f"x{c}")
                eng.dma_start(xc[:], img.ap()[c])
                X[c] = xc

            A = [None] * C
            S_ch = [None] * C

            def emit_sums(c):
                a = A[c]
                p01 = work.tile([HH, W], BF16, tag=f"s01_{c}")
                p23 = work.tile([HH, W], BF16, tag=f"s23_{c}")
                p45 = work.tile([HH, W], BF16, tag=f"s45_{c}")
                p67 = work.tile([HH, W], BF16, tag=f"s67_{c}")
                nc.vector.tensor_add(p01[:], a[:, 0, :], a[:, 1, :])
                nc.vector.tensor_add(p23[:], a[:, 2, :], a[:, 3, :])
                nc.vector.tensor_add(p45[:], a[:, 4, :], a[:, 5, :])
                nc.vector.tensor_add(p67[:], a[:, 6, :], a[:, 7, :])
                nc.vector.tensor_add(p01[:], p01[:], p23[:])
                nc.vector.tensor_add(p45[:], p45[:], p67[:])
                sc = work.tile([HH, W], F32, tag=f"s_{c}")
                nc.vector.tensor_add(sc[:], p01[:], p45[:])
                S_ch[c] = sc

            # --- x-channel exp argument on DVE/GpSimd -------------------
            # t_x[i] = 2500*ax_i*x - 1250*x^2 - 1250*ax_i^2
            px = work.tile([HH, W], F32, tag="px")
            nc.vector.scalar_tensor_tensor(
                px[:], X[0][:], -LAM, X[0][:], ALU.mult, ALU.mult
            )
            tx = work.tile([HH, BINS, W], F32, tag="tx")
            for i in range(BINS):
                nc.vector.scalar_tensor_tensor(
                    tx[:, i, :], X[0][:], 2.0 * LAM * float(ax[i]), px[:],
                    ALU.mult, ALU.add,
                )
            for i in range(BINS):
                nc.gpsimd.tensor_scalar_add(
                    tx[:, i, :], tx[:, i, :], -LAM * float(ax[i]) ** 2
                )

            # --- ScalarE: squares+exp for y,z; batched exp for x --------
            for c in (1, 2):
                t = work.tile([HH, BINS, W], F32, tag=f"t{c}")
                for i in range(BINS):
                    nc.scalar.activation(
                        t[:, i, :], X[c][:], AF.Square,
                        bias=cb[:, c * BINS + i : c * BINS + i + 1],
                        scale=s,
                    )
                a = work.tile([HH, BINS, W], BF16, tag=f"a{c}")
                nc.scalar.activation(a[:], t[:], AF.Exp, scale=-1.0)
                A[c] = a
                emit_sums(c)
                if c == 1:
                    syz = work.tile([HH, W], F32, tag="syz")

            # exp_x writes [f, i]-major directly (strided ACT store) so
            # the A2 product and LDWEIGHTS read contiguously.
            ax2 = work.tile([HH, W, BINS], BF16, tag="ax2")
            ax2_out = bass.AP(
                tensor=ax2.tensor, offset=ax2.offset,
                ap=[ax2.ap[0], [1, BINS], [BINS, W]],
            )
            nc.scalar.activation(ax2_out, tx[:], AF.Exp, scale=1.0)

            # SY*SZ as soon as both exist
            nc.vector.tensor_mul(syz[:], S_ch[1][:], S_ch[2][:])

            # --- B = Ay (x) Az, 2x-mode layout [jl, f], per-chunk tiles -
            ay_t, az_t = A[1], A[2]
            b_ch = []
            for ch in range(NCHB):
                b_ch.append(work.tile([HH, BINS * BINS, FCHB], BF16,
                                      tag=f"b{ch}", name=f"b{ch}"))

            def emit_b_chunk(ch):
                f0 = ch * FCHB
                bt = b_ch[ch]
                out_ap = bass.AP(
                    tensor=bt.tensor, offset=bt.offset,
                    ap=[bt.ap[0], [FCHB, BINS * BINS], [1, FCHB]],
                )
                ay_b = bass.AP(
                    tensor=ay_t.tensor, offset=ay_t.offset + f0,
                    ap=[ay_t.ap[0], [W, BINS], [0, BINS], [1, FCHB]],
                )
                az_b = bass.AP(
                    tensor=az_t.tensor, offset=az_t.offset + f0,
                    ap=[az_t.ap[0], [0, BINS], [W, BINS], [1, FCHB]],
                )
                nc.vector.tensor_mul(out_ap, ay_b, az_b)

            for ch in range(NCHB):
                emit_b_chunk(ch)

            # --- x sum (innermost reduce), r = 1/(S+1e-8) ---------------
            with tc.high_priority():
                sx = work.tile([HH, W], F32, tag="s_0")
                nc.vector.tensor_reduce(sx[:], ax2[:], mybir.AxisListType.X,
                                        ALU.add)
                stot = work.tile([HH, W], F32, tag="stot")
                nc.vector.tensor_mul(stot[:], syz[:], sx[:])
                nc.gpsimd.tensor_scalar_add(stot[:], stot[:], 1e-8)
                rr = work.tile([HH, W], F32, tag="rr")
                nc.vector.reciprocal_approx_fast(rr[:], stot[:])
                rb = work.tile([HH, W], BF16, tag="rb")
                nc.gpsimd.tensor_copy(rb[:], rr[:])

                # A2[f, i] = Ax_i * r; fully contiguous out/in0
                a2_ch = []
                for ch in range(NCHB):
                    a2_ch.append(work.tile([HH, FCHB, BINS], BF16,
                                           tag=f"a2_{ch}", name=f"a2_{ch}"))
                for ch in range(NCHB):
                    f0 = ch * FCHB
                    at = a2_ch[ch]
                    a2_out = bass.AP(
                        tensor=at.tensor, offset=at.offset,
                        ap=[at.ap[0], [1, FCHB * BINS]],
                    )
                    ax_b = bass.AP(
                        tensor=ax2.tensor, offset=ax2.offset + f0 * BINS,
                        ap=[ax2.ap[0], [1, FCHB * BINS]],
                    )
                    rb_b = bass.AP(
                        tensor=rb.tensor, offset=rb.offset + f0,
                        ap=[rb.ap[0], [1, FCHB], [0, BINS]],
                    )
                    nc.vector.tensor_mul(a2_out, ax_b, rb_b)

            # --- hist strips: 4 col-packed concurrent matmul streams ----
            hp = psum.tile([HH, BINS * BINS], F32, tag="hp")

            def emit_mm_chunk(ch):
                at, bt = a2_ch[ch], b_ch[ch]
                for f4 in range(0, FCHB, NPACK):
                    for g in range(NPACK):
                        fo = f4 + g
                        f = ch * FCHB + fo
                        nc.tensor.matmul(
                            hp[32 * g : 32 * g + BINS, :],
                            lhsT=at[:, fo, :], rhs=bt[:, :, fo],
                            start=(f < NPACK), stop=(f >= W - NPACK),
                            tile_position=(0, 32 * g),
                            skip_group_check=True,
                        )

            for ch in range(NCHB):
                emit_mm_chunk(ch)

            for g in range(NPACK):
                nc.vector.tensor_copy(
                    hs[32 * g : 32 * g + BINS, :],
                    hp[32 * g : 32 * g + BINS, :],
                )
            nc.sync.dma_start(hist_out.ap(), hs[:])

    nc.compile()
    return nc


# --------------------------------------------------------------------------
# dense program: arbitrary bin_centers
# --------------------------------------------------------------------------
def _build_dense():
    nc = _make_nc()
    xs = nc.dram_tensor("xs", [4, NPIX], F32, kind="ExternalInput")
    caug = nc.dram_tensor("caug", [4, K], F32, kind="ExternalInput")
    hist_out = nc.dram_tensor("hist", [1, K], F32, kind="ExternalOutput")

    with tile.TileContext(nc) as tc:
        with (
            tc.tile_pool(name="const", bufs=1) as const,
            tc.tile_pool(name="wp", bufs=3) as wp,
            tc.tile_pool(name="sp", bufs=4) as sp,
            tc.tile_pool(name="psum", bufs=3, space="PSUM") as psum,
            tc.tile_pool(name="hpsum", bufs=1, space="PSUM") as hpsum,
        ):
            xs_t = const.tile([4, NPIX], F32, tag="xs")
            nc.sync.dma_start(xs_t[:], xs.ap())
            ca = const.tile([4, K], F32, tag="ca")
            nc.sync.dma_start(ca[:], caug.ap())

            # bias = -1250*(x^2+y^2+z^2) in image layout (128 rows, 256 cols)
            X = []
            for c in range(C):
                xc = const.tile([HH, W], F32, tag=f"im{c}")
                nc.sync.dma_start(
                    xc[:], xs.ap()[c].rearrange("(p f) -> p f", p=HH)
                )
                X.append(xc)
            p2 = const.tile([HH, W], F32, tag="p2")
            tmp = const.tile([HH, W], F32, tag="p2tmp")
            nc.vector.scalar_tensor_tensor(
                p2[:], X[0][:], -LAM, X[0][:], ALU.mult, ALU.mult
            )
            nc.vector.scalar_tensor_tensor(
                tmp[:], X[1][:], -LAM, X[1][:], ALU.mult, ALU.mult
            )
            nc.vector.tensor_add(p2[:], p2[:], tmp[:])
            nc.vector.scalar_tensor_tensor(
                tmp[:], X[2][:], -LAM, X[2][:], ALU.mult, ALU.mult
            )
            nc.vector.tensor_add(p2[:], p2[:], tmp[:])

            # pixel tile j = image column j (128 pixels, stride W in xs)
            xsr = xs_t[:].rearrange("c (p f) -> c f p", f=W)

            hp = hpsum.tile([1, K], F32, tag="hp")
            prev = None  # deferred hist matmul for software pipelining
            for j in range(NT):
                u = psum.tile([HH, K], F32, tag="u")
                nc.tensor.matmul(u[:], lhsT=xsr[:, j, :], rhs=ca[:],
                                 start=True, stop=True)
                w = wp.tile([HH, K], BF16, tag="w")
                ssum = sp.tile([HH, 1], F32, tag="ssum")
                nc.scalar.activation(
                    w[:], u[:], AF.Exp,
                    bias=p2[:, j : j + 1], scale=1.0, accum_out=ssum[:],
                )
                rcol = sp.tile([HH, 1], F32, tag="rcol")
                nc.vector.tensor_scalar_add(rcol[:], ssum[:], 1e-8)
                nc.vector.reciprocal(rcol[:], rcol[:])
                rb = sp.tile([HH, 1], BF16, tag="rb")
                nc.vector.tensor_copy(rb[:], rcol[:])
                if prev is not None:
                    pw, prb, pj = prev
                    nc.tensor.matmul(hp[:], lhsT=prb[:], rhs=pw[:],
                                     start=(pj == 0), stop=False)
                prev = (w, rb, j)
            pw, prb, pj = prev
            nc.tensor.matmul(hp[:], lhsT=prb[:], rhs=pw[:],
                             start=False, stop=True)

            hs = sp.tile([1, K], F32, tag="hs")
            nc.vector.tensor_copy(hs[:], hp[:])
            nc.sync.dma_start(hist_out.ap(), hs[:])

    nc.compile()
    return nc


# --------------------------------------------------------------------------
# host orchestration
# --------------------------------------------------------------------------
_PROGRAM_CACHE = {}
_LAST_RUN = None


def _grid_axes(bin_centers):
    """Return (ax, ay, az) if bin_centers is an ij-ordered meshgrid, else None."""
    if bin_centers.shape != (K, 3):
        return None
    c3 = bin_centers.reshape(BINS, BINS, BINS, 3)
    if (
        np.all(c3[..., 0] == c3[:, :1, :1, 0])
        and np.all(c3[..., 1] == c3[:1, :, :1, 1])
        and np.all(c3[..., 2] == c3[:1, :1, :, 2])
    ):
        return (
            c3[:, 0, 0, 0].astype(np.float64),
            c3[0, :, 0, 1].astype(np.float64),
            c3[0, 0, :, 2].astype(np.float64),
        )
    return None


def kernel(x, bin_centers):
    x = np.asarray(x, dtype=np.float32)
    bin_centers = np.asarray(bin_centers, dtype=np.float32)
    assert x.shape == (B, C, H, W) and bin_centers.shape == (K, 3)

    axes = _grid_axes(bin_centers)

    if axes is not None:
        key = ("sep", bin_centers.tobytes())
        if key not in _PROGRAM_CACHE:
            _PROGRAM_CACHE[key] = _build_separable(*axes)
        nc = _PROGRAM_CACHE[key]
        s = math.sqrt(LAM)
        cb = np.concatenate(
            [-s * a for a in axes] + [-LAM * axes[0] ** 2]
        ).astype(np.float32)[None]
        in_maps = []
        for core in range(N_CORES):
            b, half = divmod(core, 2)
            shard = np.ascontiguousarray(x[b, :, half * HH : (half + 1) * HH, :])
            in_maps.append({"img": shard, "cbias": cb})
    else:
        key = ("dense",)
        if key not in _PROGRAM_CACHE:
            _PROGRAM_CACHE[key] = _build_dense()
        nc = _PROGRAM_CACHE[key]
        c64 = bin_centers.astype(np.float64)
        caug = np.concatenate(
            [2.0 * LAM * c64.T, -LAM * (c64**2).sum(1)[None]], axis=0
        ).astype(np.float32)
        ones = np.ones((1, NPIX), np.float32)
        in_maps = []
        for core in range(N_CORES):
            b, half = divmod(core, 2)
            shard = x[b, :, half * HH : (half + 1) * HH, :].reshape(C, NPIX)
            in_maps.append(
                {"xs": np.concatenate([shard, ones], 0), "caug": caug}
            )

    global _LAST_RUN
    _LAST_RUN = (nc, in_maps)
    res = run_bass_kernel_spmd(nc, in_maps, core_ids=list(range(N_CORES)))

    def _part(h):
        h = np.asarray(h, dtype=np.float32)
        if h.shape == (HH, BINS * BINS):  # separable: sum the 4 psum strips
            strips = [h[32 * g : 32 * g + BINS] for g in range(4)]
            return (strips[0] + strips[1] + strips[2] + strips[3]).reshape(K)
        return h.reshape(K)

    parts = [_part(res.results[i]["hist"]) for i in range(N_CORES)]
    hist = np.stack([parts[2 * b] + parts[2 * b + 1] for b in range(B)], 0)
    hist = hist / (hist.sum(axis=1, keepdims=True) + np.float32(1e-8))
    return hist.astype(np.float32)


# revision 25
# speedup vs baseline: 1.6548x; 1.6548x over previous
"""Trainium2 Bass kernel for nn_DifferentiableHistogram.

reference:
    px      = x.transpose(0,2,3,1).reshape(B, N, 3)           # B=4, N=65536
    dist_sq = ||px - c||^2  for K=512 bin centers             # (B,N,K)
    w       = exp(-dist_sq / (2*0.02^2))                      # = exp(-1250*d^2)
    w       = w / (sum_k w + 1e-8)
    hist    = sum_n w;  hist = hist / (sum_k hist + 1e-8)     # (B,K)

Sharding: 8 cores = 4 batches x 2 image-row halves. Each core computes a
partial unnormalized histogram over its 32768 pixels (per-pixel softmax
weights are independent across pixels); host sums the two halves per batch
and applies the final K-normalization.

Two device programs:
  * separable — when bin_centers is a 8x8x8 meshgrid (the reference's
    setup_inputs), exp factorizes per axis: w = wx_i * wy_j * wz_l. Only
    24 exps/pixel instead of 512, and the histogram becomes a 3-way
    contraction done on the tensor engine.
  * dense — any bin_centers: dist^2 via matmul (contract over the
    4-vector [x,y,z,1]), exp+row-sum fused on the scalar engine
    (accum_out), per-pixel 1/(S+1e-8) on vector, histogram accumulation
    as a second matmul contracting over pixels.
"""

import math

import numpy as np

import concourse.bass as bass
import concourse.tile as tile
from concourse import bacc, mybir
from concourse.bass_utils import run_bass_kernel_spmd

F32 = mybir.dt.float32
BF16 = mybir.dt.bfloat16
AF = mybir.ActivationFunctionType
ALU = mybir.AluOpType

B, C, H, W = 4, 3, 256, 256
BINS = 8
K = BINS**3
SIGMA = 0.02
LAM = 1.0 / (2.0 * SIGMA**2)  # 1250.0
N_CORES = 8
HH = H // 2          # rows per core: 128
NPIX = HH * W        # pixels per core: 32768
NT = W               # pixel tiles per core (one image column each): 256


def _make_nc():
    return bacc.Bacc("TRN2", target_bir_lowering=False, debug=False)


# --------------------------------------------------------------------------
# separable program: bin_centers = meshgrid(ax, ay, az), K = 8*8*8
# --------------------------------------------------------------------------
def _build_separable(ax, ay, az):
    nc = _make_nc()
    img = nc.dram_tensor("img", [C, HH, W], F32, kind="ExternalInput")
    cbias = nc.dram_tensor("cbias", [1, 4 * BINS], F32, kind="ExternalInput")
    hist_out = nc.dram_tensor("hist", [HH, BINS * BINS], F32,
                              kind="ExternalOutput")

    NCHB = 8                   # chunks over image columns
    FCHB = W // NCHB
    NPACK = 4                  # column-packed concurrent matmuls
    s = math.sqrt(LAM)
    axes = [ax, ay, az]

    with tile.TileContext(nc) as tc:
        with (
            tc.tile_pool(name="const", bufs=1) as const,
            tc.tile_pool(name="work", bufs=1) as work,
            tc.tile_pool(name="psum", bufs=1, space="PSUM") as psum,
        ):
            # bias columns first ([c*8+i] = -s*a_i for the Square path),
            # then a tiny Square to pull the ACT table load forward.
            # channels in image layout (rows x cols), split across queues
            X = [None] * C
            cb = const.tile([HH, 4 * BINS], F32, tag="cb")
            cb_src = bass.AP(
                tensor=cbias.ap().tensor, offset=0,
                ap=[[0, HH], [1, 4 * BINS]],
            )
            for c, eng in ((1, nc.sync), (0, nc.gpsimd), (2, nc.sync)):
                xc = const.tile([HH, W], F32, tag=f"x{c}")
                eng.dma_start(xc[:], img.ap()[c])
                X[c] = xc
                if c == 1:
                    nc.gpsimd.dma_start(cb[:], cb_src)
            warm = work.tile([HH, 1], F32, tag="warm")
            nc.scalar.activation(warm[:], cb[:, 0:1], AF.Square)
            hs = work.tile([HH, BINS * BINS], F32, tag="hs")
            nc.vector.memset(hs[:], 0.0)

            A = [None] * C
            S_ch = [None] * C

            def emit_sums(c):
                a = A[c]
                p01 = work.tile([HH, W], BF16, tag=f"s01_{c}")
                p23 = work.tile([HH, W], BF16, tag=f"s23_{c}")
                p45 = work.tile([HH, W], BF16, tag=f"s45_{c}")
                p67 = work.tile([HH, W], BF16, tag=f"s67_{c}")
                nc.vector.tensor_add(p01[:], a[:, 0, :], a[:, 1, :])
                nc.vector.tensor_add(p23[:], a[:, 2, :], a[:, 3, :])
                nc.vector.tensor_add(p45[:], a[:, 4, :], a[:, 5, :])
                nc.vector.tensor_add(p67[:], a[:, 6, :], a[:, 7, :])
                nc.vector.tensor_add(p01[:], p01[:], p23[:])
                nc.vector.tensor_add(p45[:], p45[:], p67[:])
                sc = work.tile([HH, W], F32, tag=f"s_{c}")
                nc.vector.tensor_add(sc[:], p01[:], p45[:])
                S_ch[c] = sc

            # --- x-channel exp argument on DVE/GpSimd -------------------
            # t_x[i] = 2500*ax_i*x - 1250*x^2 - 1250*ax_i^2
            px = work.tile([HH, W], F32, tag="px")
            nc.vector.scalar_tensor_tensor(
                px[:], X[0][:], -LAM, X[0][:], ALU.mult, ALU.mult
            )
            tx = work.tile([HH, BINS, W], F32, tag="tx")
            for i in range(BINS):
                nc.vector.scalar_tensor_tensor(
                    tx[:, i, :], X[0][:], 2.0 * LAM * float(ax[i]), px[:],
                    ALU.mult, ALU.add,
                )
            for i in range(BINS):
                nc.vector.tensor_scalar_add(
                    tx[:, i, :], tx[:, i, :], -LAM * float(ax[i]) ** 2
                )

            # --- ScalarE: squares+exp for y,z; batched exp for x --------
            for c in (1, 2):
                t = work.tile([HH, BINS, W], F32, tag=f"t{c}")
                for i in range(BINS):
                    nc.scalar.activation(
                        t[:, i, :], X[c][:], AF.Square,
                        bias=cb[:, c * BINS + i : c * BINS + i + 1],
                        scale=s,
                    )
                a = work.tile([HH, BINS, W], BF16, tag=f"a{c}")
                nc.scalar.activation(a[:], t[:], AF.Exp, scale=-1.0)
                A[c] = a
                emit_sums(c)
                if c == 1:
                    syz = work.tile([HH, W], F32, tag="syz")

            ax_t = work.tile([HH, BINS, W], BF16, tag="ax_t")
            nc.scalar.activation(ax_t[:], tx[:], AF.Exp, scale=1.0)
            A[0] = ax_t

            # SY*SZ as soon as both exist
            nc.vector.tensor_mul(syz[:], S_ch[1][:], S_ch[2][:])

            # --- B = Ay (x) Az, 2x-mode layout [jl, f], per-chunk tiles -
            ay_t, az_t = A[1], A[2]
            b_ch = []
            for ch in range(NCHB):
                b_ch.append(work.tile([HH, BINS * BINS, FCHB], BF16,
                                      tag=f"b{ch}", name=f"b{ch}"))

            def emit_b_chunk(ch):
                f0 = ch * FCHB
                bt = b_ch[ch]
                out_ap = bass.AP(
                    tensor=bt.tensor, offset=bt.offset,
                    ap=[bt.ap[0], [FCHB, BINS * BINS], [1, FCHB]],
                )
                ay_b = bass.AP(
                    tensor=ay_t.tensor, offset=ay_t.offset + f0,
                    ap=[ay_t.ap[0], [W, BINS], [0, BINS], [1, FCHB]],
                )
                az_b = bass.AP(
                    tensor=az_t.tensor, offset=az_t.offset + f0,
                    ap=[az_t.ap[0], [0, BINS], [W, BINS], [1, FCHB]],
                )
                nc.vector.tensor_mul(out_ap, ay_b, az_b)

            for ch in range(NCHB):
                emit_b_chunk(ch)

            # --- x sums, r = 1/(S+1e-8), A2 = Ax*r (one 2x op) ----------
            with tc.high_priority():
                emit_sums(0)
                stot = work.tile([HH, W], F32, tag="stot")
                nc.vector.tensor_mul(stot[:], syz[:], S_ch[0][:])
                nc.vector.tensor_scalar_add(stot[:], stot[:], 1e-8)
                rr = work.tile([HH, W], F32, tag="rr")
                nc.vector.reciprocal_approx_fast(rr[:], stot[:])
                rb = work.tile([HH, W], BF16, tag="rb")
                nc.vector.tensor_copy(rb[:], rr[:])

                # A2 in [i, f] layout: all APs innermost-contiguous -> 2x.
                # The strided LDWEIGHTS read (8 x 2B at 512B) is hidden
                # behind the matmul stream.
                a2t = work.tile([HH, BINS, W], BF16, tag="a2t")
                rb_b = bass.AP(
                    tensor=rb.tensor, offset=rb.offset,
                    ap=[rb.ap[0], [0, BINS], [1, W]],
                )
                nc.vector.tensor_mul(a2t[:], ax_t[:], rb_b)

            # --- hist strips: 4 col-packed concurrent matmul streams ----
            hp = psum.tile([HH, BINS * BINS], F32, tag="hp")

            def emit_mm_chunk(ch):
                bt = b_ch[ch]
                for f4 in range(0, FCHB, NPACK):
                    for g in range(NPACK):
                        fo = f4 + g
                        f = ch * FCHB + fo
                        nc.tensor.matmul(
                            hp[32 * g : 32 * g + BINS, :],
                            lhsT=a2t[:, :, f], rhs=bt[:, :, fo],
                            start=(f < NPACK), stop=(f >= W - NPACK),
                            tile_position=(0, 32 * g),
                            skip_group_check=True,
                        )

            for ch in range(NCHB):
                emit_mm_chunk(ch)

            for g in range(NPACK):
                nc.vector.tensor_copy(
                    hs[32 * g : 32 * g + BINS, :],
                    hp[32 * g : 32 * g + BINS, :],
                )
            nc.sync.dma_start(hist_out.ap(), hs[:])

    nc.compile()
    return nc


# --------------------------------------------------------------------------
# dense program: arbitrary bin_centers
# --------------------------------------------------------------------------
def _build_dense():
    nc = _make_nc()
    xs = nc.dram_tensor("xs", [4, NPIX], F32, kind="ExternalInput")
    caug = nc.dram_tensor("caug", [4, K], F32, kind="ExternalInput")
    hist_out = nc.dram_tensor("hist", [1, K], F32, kind="ExternalOutput")

    with tile.TileContext(nc) as tc:
        with (
            tc.tile_pool(name="const", bufs=1) as const,
            tc.tile_pool(name="wp", bufs=3) as wp,
            tc.tile_pool(name="sp", bufs=4) as sp,
            tc.tile_pool(name="psum", bufs=3, space="PSUM") as psum,
            tc.tile_pool(name="hpsum", bufs=1, space="PSUM") as hpsum,
        ):
            xs_t = const.tile([4, NPIX], F32, tag="xs")
            nc.sync.dma_start(xs_t[:], xs.ap())
            ca = const.tile([4, K], F32, tag="ca")
            nc.sync.dma_start(ca[:], caug.ap())

            # bias = -1250*(x^2+y^2+z^2) in image layout (128 rows, 256 cols)
            X = []
            for c in range(C):
                xc = const.tile([HH, W], F32, tag=f"im{c}")
                nc.sync.dma_start(
                    xc[:], xs.ap()[c].rearrange("(p f) -> p f", p=HH)
                )
                X.append(xc)
            p2 = const.tile([HH, W], F32, tag="p2")
            tmp = const.tile([HH, W], F32, tag="p2tmp")
            nc.vector.scalar_tensor_tensor(
                p2[:], X[0][:], -LAM, X[0][:], ALU.mult, ALU.mult
            )
            nc.vector.scalar_tensor_tensor(
                tmp[:], X[1][:], -LAM, X[1][:], ALU.mult, ALU.mult
            )
            nc.vector.tensor_add(p2[:], p2[:], tmp[:])
            nc.vector.scalar_tensor_tensor(
                tmp[:], X[2][:], -LAM, X[2][:], ALU.mult, ALU.mult
            )
            nc.vector.tensor_add(p2[:], p2[:], tmp[:])

            # pixel tile j = image column j (128 pixels, stride W in xs)
            xsr = xs_t[:].rearrange("c (p f) -> c f p", f=W)

            hp = hpsum.tile([1, K], F32, tag="hp")
            prev = None  # deferred hist matmul for software pipelining
            for j in range(NT):
                u = psum.tile([HH, K], F32, tag="u")
                nc.tensor.matmul(u[:], lhsT=xsr[:, j, :], rhs=ca[:],
                                 start=True, stop=True)
                w = wp.tile([HH, K], BF16, tag="w")
                ssum = sp.tile([HH, 1], F32, tag="ssum")
                nc.scalar.activation(
                    w[:], u[:], AF.Exp,
                    bias=p2[:, j : j + 1], scale=1.0, accum_out=ssum[:],
                )
                rcol = sp.tile([HH, 1], F32, tag="rcol")
                nc.vector.tensor_scalar_add(rcol[:], ssum[:], 1e-8)
                nc.vector.reciprocal(rcol[:], rcol[:])
                rb = sp.tile([HH, 1], BF16, tag="rb")
                nc.vector.tensor_copy(rb[:], rcol[:])
                if prev is not None:
                    pw, prb, pj = prev
                    nc.tensor.matmul(hp[:], lhsT=prb[:], rhs=pw[:],
                                     start=(pj == 0), stop=False)
                prev = (w, rb, j)
            pw, prb, pj = prev
            nc.tensor.matmul(hp[:], lhsT=prb[:], rhs=pw[:],
                             start=False, stop=True)

            hs = sp.tile([1, K], F32, tag="hs")
            nc.vector.tensor_copy(hs[:], hp[:])
            nc.sync.dma_start(hist_out.ap(), hs[:])

    nc.compile()
    return nc


# --------------------------------------------------------------------------
# host orchestration
# --------------------------------------------------------------------------
_PROGRAM_CACHE = {}
_LAST_RUN = None


def _grid_axes(bin_centers):
    """Return (ax, ay, az) if bin_centers is an ij-ordered meshgrid, else None."""
    if bin_centers.shape != (K, 3):
        return None
    c3 = bin_centers.reshape(BINS, BINS, BINS, 3)
    if (
        np.all(c3[..., 0] == c3[:, :1, :1, 0])
        and np.all(c3[..., 1] == c3[:1, :, :1, 1])
        and np.all(c3[..., 2] == c3[:1, :1, :, 2])
    ):
        return (
            c3[:, 0, 0, 0].astype(np.float64),
            c3[0, :, 0, 1].astype(np.float64),
            c3[0, 0, :, 2].astype(np.float64),
        )
    return None


def kernel(x, bin_centers):
    x = np.asarray(x, dtype=np.float32)
    bin_centers = np.asarray(bin_centers, dtype=np.float32)
    assert x.shape == (B, C, H, W) and bin_centers.shape == (K, 3)

    axes = _grid_axes(bin_centers)

    if axes is not None:
        key = ("sep", bin_centers.tobytes())
        if key not in _PROGRAM_CACHE:
            _PROGRAM_CACHE[key] = _build_separable(*axes)
        nc = _PROGRAM_CACHE[key]
        s = math.sqrt(LAM)
        cb = np.concatenate(
            [-s * a for a in axes] + [-LAM * axes[0] ** 2]
        ).astype(np.float32)[None]
        in_maps = []
        for core in range(N_CORES):
            b, half = divmod(core, 2)
            shard = np.ascontiguousarray(x[b, :, half * HH : (half + 1) * HH, :])
            in_maps.append({"img": shard, "cbias": cb})
    else:
        key = ("dense",)
        if key not in _PROGRAM_CACHE:
            _PROGRAM_CACHE[key] = _build_dense()
        nc = _PROGRAM_CACHE[key]
        c64 = bin_centers.astype(np.float64)
        caug = np.concatenate(
            [2.0 * LAM * c64.T, -LAM * (c64**2).sum(1)[None]], axis=0
        ).astype(np.float32)
        ones = np.ones((1, NPIX), np.float32)
        in_maps = []
        for core in range(N_CORES):
            b, half = divmod(core, 2)
            shard = x[b, :, half * HH : (half + 1) * HH, :].reshape(C, NPIX)
            in_maps.append(
                {"xs": np.concatenate([shard, ones], 0), "caug": caug}
            )

    global _LAST_RUN
    _LAST_RUN = (nc, in_maps)
    res = run_bass_kernel_spmd(nc, in_maps, core_ids=list(range(N_CORES)))

    def _part(h):
        h = np.asarray(h, dtype=np.float32)
        if h.shape == (HH, BINS * BINS):  # separable: sum the 4 psum strips
            strips = [h[32 * g : 32 * g + BINS] for g in range(4)]
            return (strips[0] + strips[1] + strips[2] + strips[3]).reshape(K)
        return h.reshape(K)

    parts = [_part(res.results[i]["hist"]) for i in range(N_CORES)]
    hist = np.stack([parts[2 * b] + parts[2 * b + 1] for b in range(B)], 0)
    hist = hist / (hist.sum(axis=1, keepdims=True) + np.float32(1e-8))
    return hist.astype(np.float32)


# revision 26
# speedup vs baseline: 1.6883x; 1.0202x over previous
"""Trainium2 Bass kernel for nn_DifferentiableHistogram.

reference:
    px      = x.transpose(0,2,3,1).reshape(B, N, 3)           # B=4, N=65536
    dist_sq = ||px - c||^2  for K=512 bin centers             # (B,N,K)
    w       = exp(-dist_sq / (2*0.02^2))                      # = exp(-1250*d^2)
    w       = w / (sum_k w + 1e-8)
    hist    = sum_n w;  hist = hist / (sum_k hist + 1e-8)     # (B,K)

Sharding: 8 cores = 4 batches x 2 image-row halves. Each core computes a
partial unnormalized histogram over its 32768 pixels (per-pixel softmax
weights are independent across pixels); host sums the two halves per batch
and applies the final K-normalization.

Two device programs:
  * separable — when bin_centers is a 8x8x8 meshgrid (the reference's
    setup_inputs), exp factorizes per axis: w = wx_i * wy_j * wz_l. Only
    24 exps/pixel instead of 512, and the histogram becomes a 3-way
    contraction done on the tensor engine.
  * dense — any bin_centers: dist^2 via matmul (contract over the
    4-vector [x,y,z,1]), exp+row-sum fused on the scalar engine
    (accum_out), per-pixel 1/(S+1e-8) on vector, histogram accumulation
    as a second matmul contracting over pixels.
"""

import math

import numpy as np

import concourse.bass as bass
import concourse.tile as tile
from concourse import bacc, mybir
from concourse.bass_utils import run_bass_kernel_spmd

F32 = mybir.dt.float32
BF16 = mybir.dt.bfloat16
AF = mybir.ActivationFunctionType
ALU = mybir.AluOpType

B, C, H, W = 4, 3, 256, 256
BINS = 8
K = BINS**3
SIGMA = 0.02
LAM = 1.0 / (2.0 * SIGMA**2)  # 1250.0
N_CORES = 8
HH = H // 2          # rows per core: 128
NPIX = HH * W        # pixels per core: 32768
NT = W               # pixel tiles per core (one image column each): 256


def _make_nc():
    return bacc.Bacc("TRN2", target_bir_lowering=False, debug=False)


# --------------------------------------------------------------------------
# separable program: bin_centers = meshgrid(ax, ay, az), K = 8*8*8
# --------------------------------------------------------------------------
def _build_separable(ax, ay, az):
    nc = _make_nc()
    img = nc.dram_tensor("img", [C, HH, W], F32, kind="ExternalInput")
    cbias = nc.dram_tensor("cbias", [1, 4 * BINS], F32, kind="ExternalInput")
    hist_out = nc.dram_tensor("hist", [HH, BINS * BINS], F32,
                              kind="ExternalOutput")

    NCHB = 8                   # chunks over image columns
    FCHB = W // NCHB
    NPACK = 4                  # column-packed concurrent matmuls
    s = math.sqrt(LAM)
    axes = [ax, ay, az]

    with tile.TileContext(nc) as tc:
        with (
            tc.tile_pool(name="const", bufs=1) as const,
            tc.tile_pool(name="work", bufs=1) as work,
            tc.tile_pool(name="psum", bufs=1, space="PSUM") as psum,
        ):
            # bias columns first ([c*8+i] = -s*a_i for the Square path),
            # then a tiny Square to pull the ACT table load forward.
            # channels in image layout (rows x cols), split across queues;
            # warm-up Square reads a preloaded const so the ACT table load
            # never waits on a DMA.
            warm = work.tile([HH, 1], F32, tag="warm")
            nc.scalar.activation(warm[:], nc.const_aps.aps[(F32, 1.0)][:HH],
                                 AF.Square)
            X = [None] * C
            cb = const.tile([HH, 4 * BINS], F32, tag="cb")
            cb_src = bass.AP(
                tensor=cbias.ap().tensor, offset=0,
                ap=[[0, HH], [1, 4 * BINS]],
            )
            nc.gpsimd.dma_start(cb[:], cb_src)
            for c, eng in ((1, nc.sync), (0, nc.gpsimd), (2, nc.sync)):
                xc = const.tile([HH, W], F32, tag=f"x{c}")
                eng.dma_start(xc[:], img.ap()[c])
                X[c] = xc
            hs = work.tile([HH, BINS * BINS], F32, tag="hs")
            nc.vector.memset(hs[:], 0.0)

            A = [None] * C
            S_ch = [None] * C

            def emit_sums(c):
                a = A[c]
                p01 = work.tile([HH, W], BF16, tag=f"s01_{c}")
                p23 = work.tile([HH, W], BF16, tag=f"s23_{c}")
                p45 = work.tile([HH, W], BF16, tag=f"s45_{c}")
                p67 = work.tile([HH, W], BF16, tag=f"s67_{c}")
                nc.vector.tensor_add(p01[:], a[:, 0, :], a[:, 1, :])
                nc.vector.tensor_add(p23[:], a[:, 2, :], a[:, 3, :])
                nc.vector.tensor_add(p45[:], a[:, 4, :], a[:, 5, :])
                nc.vector.tensor_add(p67[:], a[:, 6, :], a[:, 7, :])
                nc.vector.tensor_add(p01[:], p01[:], p23[:])
                nc.vector.tensor_add(p45[:], p45[:], p67[:])
                sc = work.tile([HH, W], F32, tag=f"s_{c}")
                nc.vector.tensor_add(sc[:], p01[:], p45[:])
                S_ch[c] = sc

            # --- x-channel exp argument on DVE/GpSimd -------------------
            # t_x[i] = 2500*ax_i*x - 1250*x^2 - 1250*ax_i^2
            px = work.tile([HH, W], F32, tag="px")
            nc.vector.scalar_tensor_tensor(
                px[:], X[0][:], -LAM, X[0][:], ALU.mult, ALU.mult
            )
            tx = work.tile([HH, BINS, W], F32, tag="tx")
            for i in range(BINS):
                nc.vector.scalar_tensor_tensor(
                    tx[:, i, :], X[0][:], 2.0 * LAM * float(ax[i]), px[:],
                    ALU.mult, ALU.add,
                )
            for i in range(BINS):
                nc.vector.tensor_scalar_add(
                    tx[:, i, :], tx[:, i, :], -LAM * float(ax[i]) ** 2
                )

            # --- ScalarE: squares+exp for y,z; batched exp for x --------
            for c in (1, 2):
                t = work.tile([HH, BINS, W], F32, tag=f"t{c}")
                for i in range(BINS):
                    nc.scalar.activation(
                        t[:, i, :], X[c][:], AF.Square,
                        bias=cb[:, c * BINS + i : c * BINS + i + 1],
                        scale=s,
                    )
                a = work.tile([HH, BINS, W], BF16, tag=f"a{c}")
                nc.scalar.activation(a[:], t[:], AF.Exp, scale=-1.0)
                A[c] = a
                emit_sums(c)
                if c == 1:
                    syz = work.tile([HH, W], F32, tag="syz")

            ax_t = work.tile([HH, BINS, W], BF16, tag="ax_t")
            nc.scalar.activation(ax_t[:], tx[:], AF.Exp, scale=1.0)
            A[0] = ax_t

            # SY*SZ as soon as both exist
            nc.vector.tensor_mul(syz[:], S_ch[1][:], S_ch[2][:])

            # --- B = Ay (x) Az, 2x-mode layout [jl, f], per-chunk tiles -
            ay_t, az_t = A[1], A[2]
            b_ch = []
            for ch in range(NCHB):
                b_ch.append(work.tile([HH, BINS * BINS, FCHB], BF16,
                                      tag=f"b{ch}", name=f"b{ch}"))

            def emit_b_chunk(ch):
                f0 = ch * FCHB
                bt = b_ch[ch]
                out_ap = bass.AP(
                    tensor=bt.tensor, offset=bt.offset,
                    ap=[bt.ap[0], [FCHB, BINS * BINS], [1, FCHB]],
                )
                ay_b = bass.AP(
                    tensor=ay_t.tensor, offset=ay_t.offset + f0,
                    ap=[ay_t.ap[0], [W, BINS], [0, BINS], [1, FCHB]],
                )
                az_b = bass.AP(
                    tensor=az_t.tensor, offset=az_t.offset + f0,
                    ap=[az_t.ap[0], [0, BINS], [W, BINS], [1, FCHB]],
                )
                nc.vector.tensor_mul(out_ap, ay_b, az_b)

            # --- x sums, r = 1/(S+1e-8), A2 = Ax*r (one 2x op) ----------
            with tc.high_priority():
                emit_sums(0)
                stot = work.tile([HH, W], F32, tag="stot")
                nc.vector.tensor_mul(stot[:], syz[:], S_ch[0][:])
                nc.vector.tensor_scalar_add(stot[:], stot[:], 1e-8)
                rr = work.tile([HH, W], F32, tag="rr")
                nc.vector.reciprocal_approx_fast(rr[:], stot[:])
                rb = work.tile([HH, W], BF16, tag="rb")
                nc.vector.tensor_copy(rb[:], rr[:])

                # A2 in [i, f] layout: all APs innermost-contiguous -> 2x.
                # The strided LDWEIGHTS read (8 x 2B at 512B) is hidden
                # behind the matmul stream. 4 chunks for earlier matmuls.
                a2t = work.tile([HH, BINS, W], BF16, tag="a2t")
                for q in range(4):
                    fq = q * (W // 4)
                    nc.vector.tensor_mul(
                        a2t[:, :, fq : fq + W // 4],
                        ax_t[:, :, fq : fq + W // 4],
                        bass.AP(tensor=rb.tensor, offset=rb.offset + fq,
                                ap=[rb.ap[0], [0, BINS], [1, W // 4]]),
                    )

            # --- hist strips: 4 col-packed concurrent matmul streams ----
            hp = psum.tile([HH, BINS * BINS], F32, tag="hp")

            def emit_mm_chunk(ch):
                bt = b_ch[ch]
                for f4 in range(0, FCHB, NPACK):
                    for g in range(NPACK):
                        fo = f4 + g
                        f = ch * FCHB + fo
                        nc.tensor.matmul(
                            hp[32 * g : 32 * g + BINS, :],
                            lhsT=a2t[:, :, f], rhs=bt[:, :, fo],
                            start=(f < NPACK), stop=(f >= W - NPACK),
                            tile_position=(0, 32 * g),
                            skip_group_check=True,
                        )

            for ch in range(NCHB):
                emit_b_chunk(ch)
                emit_mm_chunk(ch)

            for g in range(NPACK):
                nc.vector.tensor_copy(
                    hs[32 * g : 32 * g + BINS, :],
                    hp[32 * g : 32 * g + BINS, :],
                )
            nc.sync.dma_start(hist_out.ap(), hs[:])

    nc.compile()
    return nc


# --------------------------------------------------------------------------
# dense program: arbitrary bin_centers
# --------------------------------------------------------------------------
def _build_dense():
    nc = _make_nc()
    xs = nc.dram_tensor("xs", [4, NPIX], F32, kind="ExternalInput")
    caug = nc.dram_tensor("caug", [4, K], F32, kind="ExternalInput")
    hist_out = nc.dram_tensor("hist", [1, K], F32, kind="ExternalOutput")

    with tile.TileContext(nc) as tc:
        with (
            tc.tile_pool(name="const", bufs=1) as const,
            tc.tile_pool(name="wp", bufs=3) as wp,
            tc.tile_pool(name="sp", bufs=4) as sp,
            tc.tile_pool(name="psum", bufs=3, space="PSUM") as psum,
            tc.tile_pool(name="hpsum", bufs=1, space="PSUM") as hpsum,
        ):
            xs_t = const.tile([4, NPIX], F32, tag="xs")
            nc.sync.dma_start(xs_t[:], xs.ap())
            ca = const.tile([4, K], F32, tag="ca")
            nc.sync.dma_start(ca[:], caug.ap())

            # bias = -1250*(x^2+y^2+z^2) in image layout (128 rows, 256 cols)
            X = []
            for c in range(C):
                xc = const.tile([HH, W], F32, tag=f"im{c}")
                nc.sync.dma_start(
                    xc[:], xs.ap()[c].rearrange("(p f) -> p f", p=HH)
                )
                X.append(xc)
            p2 = const.tile([HH, W], F32, tag="p2")
            tmp = const.tile([HH, W], F32, tag="p2tmp")
            nc.vector.scalar_tensor_tensor(
                p2[:], X[0][:], -LAM, X[0][:], ALU.mult, ALU.mult
            )
            nc.vector.scalar_tensor_tensor(
                tmp[:], X[1][:], -LAM, X[1][:], ALU.mult, ALU.mult
            )
            nc.vector.tensor_add(p2[:], p2[:], tmp[:])
            nc.vector.scalar_tensor_tensor(
                tmp[:], X[2][:], -LAM, X[2][:], ALU.mult, ALU.mult
            )
            nc.vector.tensor_add(p2[:], p2[:], tmp[:])

            # pixel tile j = image column j (128 pixels, stride W in xs)
            xsr = xs_t[:].rearrange("c (p f) -> c f p", f=W)

            hp = hpsum.tile([1, K], F32, tag="hp")
            prev = None  # deferred hist matmul for software pipelining
            for j in range(NT):
                u = psum.tile([HH, K], F32, tag="u")
                nc.tensor.matmul(u[:], lhsT=xsr[:, j, :], rhs=ca[:],
                                 start=True, stop=True)
                w = wp.tile([HH, K], BF16, tag="w")
                ssum = sp.tile([HH, 1], F32, tag="ssum")
                nc.scalar.activation(
                    w[:], u[:], AF.Exp,
                    bias=p2[:, j : j + 1], scale=1.0, accum_out=ssum[:],
                )
                rcol = sp.tile([HH, 1], F32, tag="rcol")
                nc.vector.tensor_scalar_add(rcol[:], ssum[:], 1e-8)
                nc.vector.reciprocal(rcol[:], rcol[:])
                rb = sp.tile([HH, 1], BF16, tag="rb")
                nc.vector.tensor_copy(rb[:], rcol[:])
                if prev is not None:
                    pw, prb, pj = prev
                    nc.tensor.matmul(hp[:], lhsT=prb[:], rhs=pw[:],
                                     start=(pj == 0), stop=False)
                prev = (w, rb, j)
            pw, prb, pj = prev
            nc.tensor.matmul(hp[:], lhsT=prb[:], rhs=pw[:],
                             start=False, stop=True)

            hs = sp.tile([1, K], F32, tag="hs")
            nc.vector.tensor_copy(hs[:], hp[:])
            nc.sync.dma_start(hist_out.ap(), hs[:])

    nc.compile()
    return nc


# --------------------------------------------------------------------------
# host orchestration
# --------------------------------------------------------------------------
_PROGRAM_CACHE = {}
_LAST_RUN = None


def _grid_axes(bin_centers):
    """Return (ax, ay, az) if bin_centers is an ij-ordered meshgrid, else None."""
    if bin_centers.shape != (K, 3):
        return None
    c3 = bin_centers.reshape(BINS, BINS, BINS, 3)
    if (
        np.all(c3[..., 0] == c3[:, :1, :1, 0])
        and np.all(c3[..., 1] == c3[:1, :, :1, 1])
        and np.all(c3[..., 2] == c3[:1, :1, :, 2])
    ):
        return (
            c3[:, 0, 0, 0].astype(np.float64),
            c3[0, :, 0, 1].astype(np.float64),
            c3[0, 0, :, 2].astype(np.float64),
        )
    return None


def kernel(x, bin_centers):
    x = np.asarray(x, dtype=np.float32)
    bin_centers = np.asarray(bin_centers, dtype=np.float32)
    assert x.shape == (B, C, H, W) and bin_centers.shape == (K, 3)

    axes = _grid_axes(bin_centers)

    if axes is not None:
        key = ("sep", bin_centers.tobytes())
        if key not in _PROGRAM_CACHE:
            _PROGRAM_CACHE[key] = _build_separable(*axes)
        nc = _PROGRAM_CACHE[key]
        s = math.sqrt(LAM)
        cb = np.concatenate(
            [-s * a for a in axes] + [-LAM * axes[0] ** 2]
        ).astype(np.float32)[None]
        in_maps = []
        for core in range(N_CORES):
            b, half = divmod(core, 2)
            shard = np.ascontiguousarray(x[b, :, half * HH : (half + 1) * HH, :])
            in_maps.append({"img": shard, "cbias": cb})
    else:
        key = ("dense",)
        if key not in _PROGRAM_CACHE:
            _PROGRAM_CACHE[key] = _build_dense()
        nc = _PROGRAM_CACHE[key]
        c64 = bin_centers.astype(np.float64)
        caug = np.concatenate(
            [2.0 * LAM * c64.T, -LAM * (c64**2).sum(1)[None]], axis=0
        ).astype(np.float32)
        ones = np.ones((1, NPIX), np.float32)
        in_maps = []
        for core in range(N_CORES):
            b, half = divmod(core, 2)
            shard = x[b, :, half * HH : (half + 1) * HH, :].reshape(C, NPIX)
            in_maps.append(
                {"xs": np.concatenate([shard, ones], 0), "caug": caug}
            )

    global _LAST_RUN
    _LAST_RUN = (nc, in_maps)
    res = run_bass_kernel_spmd(nc, in_maps, core_ids=list(range(N_CORES)))

    def _part(h):
        h = np.asarray(h, dtype=np.float32)
        if h.shape == (HH, BINS * BINS):  # separable: sum the 4 psum strips
            strips = [h[32 * g : 32 * g + BINS] for g in range(4)]
            return (strips[0] + strips[1] + strips[2] + strips[3]).reshape(K)
        return h.reshape(K)

    parts = [_part(res.results[i]["hist"]) for i in range(N_CORES)]
    hist = np.stack([parts[2 * b] + parts[2 * b + 1] for b in range(B)], 0)
    hist = hist / (hist.sum(axis=1, keepdims=True) + np.float32(1e-8))
    return hist.astype(np.float32)
